# revision 19
# baseline (speedup 1.0000x reference)
"""EEGFormer transformer-block kernel for 8 Trainium2 NeuronCores.

Strategy: pure data parallelism over the B*S = 128 independent attention
slices; each core processes 16 slices as 8 megatiles of 512 tokens.

v3 (all-bf16 matmuls, fp32 statistics/residuals):
- All transposes (LayerNorm hT/h2T and softmax pT) run on the DMA XBAR
  (dma_start_transpose), freeing ~12k PE cycles per megatile and
  removing their psum->sbuf copies entirely.
- LayerNorm gamma/beta folded into weights/biases host-side.
- rsqrt computed as exp(-0.5*ln(var+eps)): scalar engine never swaps
  activation tables (exp/ln/copy/identity/relu share one table set).
- Weight DMAs ordered behind the first x tile; FFN1 of megatile mt-1 is
  emitted in single-matmul grains to fill every attention-phase stall.
"""

import os
import sys

import numpy as np

if "/opt/trn_rl_repo" not in sys.path and os.path.isdir("/opt/trn_rl_repo"):
    sys.path.insert(0, "/opt/trn_rl_repo")

B, S, C, L = 4, 32, 256, 512
H = 8
D = L // H
FL = 4 * L
EPS = 1e-5
N_CORES = 8
SLICES = (B * S) // N_CORES       # 16 slices per core
MT_SLICES = 2                      # slices per megatile
N_MT = SLICES // MT_SLICES         # 8 megatiles
TOK = C * MT_SLICES                # 512 tokens per megatile
TC = TOK // 128                    # 4 token chunks
LC = L // 128                      # 4 feature chunks
FC = FL // 128                     # 16 ffn-hidden chunks

_cache = {}


def _build(lnt_dma=True, ptt_dma=True):
    import concourse.bacc as bacc
    import concourse.mybir as mybir
    import concourse.tile as tile
    from concourse.masks import make_identity

    f32 = mybir.dt.float32
    bf16 = mybir.dt.bfloat16
    AF = mybir.ActivationFunctionType
    OP = mybir.AluOpType

    nc = bacc.Bacc("TRN2", target_bir_lowering=False)

    x_d = nc.dram_tensor("x", [SLICES, C, L], f32, kind="ExternalInput")
    wqkv_d = nc.dram_tensor("wqkvT", [3 * L, L], bf16, kind="ExternalInput")
    wow2_d = nc.dram_tensor("wow2T", [L + FL, L], bf16, kind="ExternalInput")
    w1_d = nc.dram_tensor("w1T", [L, FL], bf16, kind="ExternalInput")
    cqk_d = nc.dram_tensor("cqk", [2, L], f32, kind="ExternalInput")
    b1_d = nc.dram_tensor("b1p", [FL], f32, kind="ExternalInput")
    bo_d = nc.dram_tensor("bop", [L], f32, kind="ExternalInput")
    b2_d = nc.dram_tensor("b2", [L], f32, kind="ExternalInput")
    out_d = nc.dram_tensor("out", [SLICES, C, L], f32, kind="ExternalOutput")

    x_v = x_d[:, :, :].rearrange("s (tc p) l -> (s tc) p l", p=128)
    out_v = out_d[:, :, :].rearrange("s (tc p) l -> (s tc) p l", p=128)

    import concourse.bass as bass

    def bcast_row(vec_ap, p=128):
        return bass.AP(
            tensor=vec_ap.tensor,
            offset=vec_ap.offset,
            ap=[[0, p]] + list(vec_ap.ap),
        )

    with tile.TileContext(nc) as tc_ctx:
        tc = tc_ctx
        import contextlib

        ctx = contextlib.ExitStack()
        with ctx:
            wpool = ctx.enter_context(tc.tile_pool(name="weights", bufs=1))
            const = ctx.enter_context(tc.tile_pool(name="const", bufs=1))
            xin = ctx.enter_context(tc.tile_pool(name="xin", bufs=2))
            act = ctx.enter_context(tc.tile_pool(name="act", bufs=2))
            sm = ctx.enter_context(tc.tile_pool(name="sm", bufs=3))
            yp = ctx.enter_context(tc.tile_pool(name="yp", bufs=2))
            outp = ctx.enter_context(tc.tile_pool(name="outp", bufs=2))
            stat = ctx.enter_context(tc.tile_pool(name="stat", bufs=12))
            natt = 3 if ptt_dma else 4
            ps_att = ctx.enter_context(tc.tile_pool(name="ps_att", bufs=natt, space="PSUM"))
            ps_ffn = ctx.enter_context(tc.tile_pool(name="ps_ffn", bufs=2, space="PSUM"))
            ps_cyc = ctx.enter_context(tc.tile_pool(name="ps_cyc", bufs=3 if ptt_dma else 2, space="PSUM"))

            ident16 = const.tile([128, 128], bf16)
            make_identity(nc, ident16)

            ones_w = const.tile([128, 128], bf16)
            nc.vector.memset(ones_w, 1.0)

            def warm_pe(n=40):
                # Dummy matmuls at t=0: keep the PE activity window busy
                # while the first DMAs + LN land so HAM un-throttles early
                # and stays warm until the first real matmul (~14us).
                wps = ps_cyc.tile([128, TOK], f32, name="warm", tag="ps_cyc")
                for _ in range(n):
                    nc.tensor.matmul(
                        wps[:, :128], ones_w, ones_w, start=True, stop=True
                    )
                nc.vector.tensor_copy(
                    out=const.tile([128, 128], f32, name="warm_sink"),
                    in_=wps[:, :128],
                )

            wqkv_s = wpool.tile([128, 3, LC, L], bf16)
            wq_s = wqkv_s[:, 0]
            wk_s = wqkv_s[:, 1]
            wv_s = wqkv_s[:, 2]
            wow2_s = wpool.tile([128, LC + FC, L], bf16)
            wo_s = wow2_s[:, :LC]
            w2_s = wow2_s[:, LC:]
            w1_s = wpool.tile([128, LC, FL], bf16)

            cqk_s = const.tile([128, 2, LC], f32)
            cq_s = cqk_s[:, 0]
            ck_s = cqk_s[:, 1]
            b1_s = const.tile([128, FC], f32)
            bo_b = const.tile([128, L], f32)
            b2_b = const.tile([128, L], f32)
            eps_c = const.tile([128, 1], f32)
            nc.vector.memset(eps_c, EPS)
            zero_c = const.tile([128, 1], f32)
            nc.vector.memset(zero_c, 0.0)

            def load_weights_early():
                nc.sync.dma_start(
                    out=wqkv_s,
                    in_=wqkv_d[:, :].rearrange("(w kc p) f -> p w kc f", p=128, w=3),
                )
                nc.sync.dma_start(
                    out=cqk_s,
                    in_=cqk_d[:, :].rearrange("w (c p) -> p w c", p=128),
                )

            def load_weights_late():
                nc.sync.dma_start(
                    out=w1_s, in_=w1_d[:, :].rearrange("(kc p) f -> p kc f", p=128)
                )
                nc.sync.dma_start(
                    out=wow2_s,
                    in_=wow2_d[:, :].rearrange("(kc p) f -> p kc f", p=128),
                )
                nc.sync.dma_start(
                    out=b1_s, in_=b1_d[:].rearrange("(c p) -> p c", p=128)
                )
                nc.gpsimd.dma_start(out=bo_b, in_=bcast_row(bo_d[:]))
                nc.gpsimd.dma_start(out=b2_b, in_=bcast_row(b2_d[:]))

            def emit_stats(x_sb, mv, bn, t):
                nc.vector.bn_stats(out=bn, in_=x_sb[:, t, :])
                nc.vector.bn_aggr(out=mv[:, t, :], in_=bn)

            def layernorm_T(x_sb, tagb, mt, ln1, mv_pre=None):
                """LN over features of x_sb [128, TC, L] fp32 (tokens on
                partitions); returns hT view [128, LC, TC, 128] (features on
                partitions, bf16, no gamma/beta: folded into weights)."""
                xcn = act.tile([128, TC, L], bf16, name=f"xcn_{tagb}_{mt}", tag=f"xcn_{tagb}", bufs=1)
                if mv_pre is not None:
                    mv = mv_pre
                else:
                    mv = stat.tile([128, TC, 2], f32, name=f"mv_{tagb}_{mt}", tag="mv")
                rstd = stat.tile([128, TC], f32, name=f"rstd_{tagb}_{mt}", tag="rstd")
                lnv = stat.tile([128, TC], f32, name=f"lnv_{tagb}_{mt}", tag="lnv")
                bn = stat.tile([128, 6], f32, name=f"bn_{tagb}_{mt}", tag="bn")
                if mv_pre is None:
                    for t in range(TC):
                        emit_stats(x_sb, mv, bn, t)
                # rstd = rsqrt(var+eps) via Newton on DVE (keeps the scalar
                # engine on a single activation table: no ACT_TABLE_LOADs).
                vv = stat.tile([128, TC], f32, name=f"vv_{tagb}_{mt}", tag="vv")
                tt = stat.tile([128, TC], f32, name=f"tt_{tagb}_{mt}", tag="tt")
                nc.vector.tensor_scalar(
                    out=vv, in0=mv[:, :, 1], scalar1=EPS, scalar2=None, op0=OP.add
                )
                # seed y0 = 2/(1+v), then 3 Newton steps y *= 1.5 - 0.5*v*y^2
                nc.vector.tensor_scalar(
                    out=lnv, in0=vv, scalar1=1.0, scalar2=None, op0=OP.add
                )
                nc.vector.reciprocal(out=lnv, in_=lnv)
                nc.vector.tensor_scalar(
                    out=rstd, in0=lnv, scalar1=2.0, scalar2=None, op0=OP.mult
                )
                for _ in range(3):
                    nc.vector.tensor_mul(out=tt, in0=rstd, in1=rstd)
                    nc.vector.tensor_mul(out=tt, in0=tt, in1=vv)
                    nc.vector.tensor_scalar(
                        out=tt, in0=tt, scalar1=-0.5, scalar2=1.5,
                        op0=OP.mult, op1=OP.add,
                    )
                    nc.vector.tensor_mul(out=rstd, in0=rstd, in1=tt)
                for t in range(TC):
                    nc.vector.tensor_scalar(
                        out=xcn[:, t, :], in0=x_sb[:, t, :],
                        scalar1=mv[:, t, 0:1], scalar2=rstd[:, t : t + 1],
                        op0=OP.subtract, op1=OP.mult,
                    )
                if lnt_dma:
                    # hTd[p, t*LC+lc, f] = xcn.T[(t*LC+lc)*128+p, f]
                    # split per t so each transpose pipelines with the LN loop
                    hTd = act.tile([128, TC * LC, 128], bf16, name=f"hT_{tagb}_{mt}", tag=f"hT_{tagb}")
                    for t in range(TC):
                        nc.sync.dma_start_transpose(
                            out=hTd[:, t * LC : (t + 1) * LC, :], in_=xcn[:, t, :]
                        )
                    return hTd[:, :, :].rearrange("p (t l) f -> p l t f", l=LC)
                hT = act.tile([128, LC, TOK], bf16, name=f"hT_{tagb}_{mt}", tag=f"hT_{tagb}")
                for m in range(LC):
                    hps = ps_cyc.tile([128, TOK], f32, name=f"hps_{tagb}_{mt}_{m}", tag="ps_cyc")
                    for t in range(TC):
                        nc.tensor.matmul(
                            hps[:, t * 128 : (t + 1) * 128],
                            xcn[:, t, m * 128 : (m + 1) * 128],
                            ident16,
                        )
                    if ln1:
                        nc.vector.tensor_copy(out=hT[:, m, :], in_=hps)
                    else:
                        nc.scalar.copy(out=hT[:, m, :], in_=hps)
                return hT[:, :, :].rearrange("p l (t f) -> p l t f", f=128)

            x_pref = {}

            def prefetch_x(mt):
                if mt in x_pref or mt >= N_MT:
                    return
                x_sb = xin.tile([128, TC, L], f32, name=f"x_{mt}", tag="x")
                if mt == 0:
                    for t in range(TC):
                        nc.scalar.dma_start(out=x_sb[:, t, :], in_=x_v[4 * mt + t])
                else:
                    nc.scalar.dma_start(
                        out=x_sb,
                        in_=x_v[4 * mt : 4 * mt + 4].rearrange("c p l -> p c l"),
                    )
                x_pref[mt] = x_sb

            def emit_ln1(mt):
                prefetch_x(mt)
                x_sb = x_pref.pop(mt)
                hT = layernorm_T(x_sb, "ln1", mt, ln1=True)
                return x_sb, hT

            def emit_qkv_units(mt, hT):
                """hT is a [128, LC, TC, 128] view. Returns tiles + closures."""
                qT = act.tile([128, LC, TOK], bf16, name=f"qT_{mt}", tag="qT", bufs=1)
                kT = act.tile([128, LC, TOK], bf16, name=f"kT_{mt}", tag="kT", bufs=1)
                v_sb = act.tile([128, TC, L], bf16, name=f"v_{mt}", tag="v", bufs=1)
                units = []
                for m in range(LC):
                    def mk_q(m=m):
                        pq = ps_cyc.tile([128, TOK], f32, name=f"psq_{mt}_{m}", tag="ps_cyc")
                        for kc in range(LC):
                            nc.tensor.matmul(
                                pq, wq_s[:, kc, m * 128 : (m + 1) * 128],
                                hT[:, kc, :, :],
                                start=(kc == 0), stop=(kc == LC - 1),
                            )
                        nc.vector.tensor_scalar(
                            out=qT[:, m, :], in0=pq,
                            scalar1=cq_s[:, m : m + 1], scalar2=None, op0=OP.add,
                        )
                    def mk_k(m=m):
                        pk = ps_cyc.tile([128, TOK], f32, name=f"psk_{mt}_{m}", tag="ps_cyc")
                        for kc in range(LC):
                            nc.tensor.matmul(
                                pk, wk_s[:, kc, m * 128 : (m + 1) * 128],
                                hT[:, kc, :, :],
                                start=(kc == 0), stop=(kc == LC - 1),
                            )
                        nc.vector.tensor_scalar(
                            out=kT[:, m, :], in0=pk,
                            scalar1=ck_s[:, m : m + 1], scalar2=None, op0=OP.add,
                        )
                    units.append(mk_q)
                    units.append(mk_k)
                vunits = []
                for t in range(TC):
                    def mk_v(t=t):
                        pv = ps_cyc.tile([128, L], f32, name=f"psv_{mt}_{t}", tag="ps_cyc")
                        for kc in range(LC):
                            nc.tensor.matmul(
                                pv, hT[:, kc, t, :], wv_s[:, kc, :],
                                start=(kc == 0), stop=(kc == LC - 1),
                            )
                        nc.scalar.copy(out=v_sb[:, t, :], in_=pv)
                    vunits.append(mk_v)
                # all q/k first, v last: v-copies of a prefetched megatile
                # must never be emitted before the current megatile's last AV
                # (scalar-queue cycle through the v ring buffer otherwise).
                units = units + vunits
                return qT, kT, v_sb, units

            def emit_attn_S(mt, qT, kT, m, sl, sps):
                tok_sl = slice(sl * C, (sl + 1) * C)
                for hh in range(2):
                    sps[hh] = ps_att.tile(
                        [128, 2, C], f32, name=f"s_{mt}_{m}_{sl}_{hh}", tag="ps_s", bufs=3
                    )
                for qc in range(2):
                    for hh in range(2):
                        prow = hh * 64
                        nc.tensor.matmul(
                            sps[hh][:, qc, :],
                            qT[prow : prow + 64, m, tok_sl][:, qc * 128 : (qc + 1) * 128],
                            kT[prow : prow + 64, m, tok_sl],
                        )

            def emit_soft(mt, m, sl, sps, pexp, z):
                """exp into pexp slice [128, (sl,hh,qc,k)], z-reduce, rz-mul."""
                for hh in range(2):
                    nc.scalar.activation(
                        out=pexp[:, sl, hh, :, :], in_=sps[hh][:, :, :], func=AF.Exp,
                        scale=float(D) ** -0.5, bias=zero_c,
                    )
                with nc.allow_low_precision(reason="softmax z: row-scale only"):
                    for hh in range(2):
                        nc.vector.tensor_reduce(
                            out=z[:, sl, hh, :], in_=pexp[:, sl, hh, :, :],
                            axis=mybir.AxisListType.X, op=OP.add,
                        )
                rz = stat.tile([128, 4], f32, name=f"rz_{mt}_{m}_{sl}", tag="rz")
                nc.vector.reciprocal(out=rz, in_=z[:, sl, :, :].rearrange("p a b -> p (a b)"))
                for hh in range(2):
                    for qc in range(2):
                        nc.vector.tensor_scalar_mul(
                            pexp[:, sl, hh, qc, :], pexp[:, sl, hh, qc, :],
                            rz[:, 2 * hh + qc : 2 * hh + qc + 1],
                        )

            def emit_pt_dma(mt, m, pexp):
                """One XBAR transpose for the whole [128, 2048] pexp of m.
                pTd[p, mm, f] = pexp[q=f, free=mm*128+p]; mm = sl*8+hh*4+qc*2+kc.
                Returns rhs_fn(sl, hh, kc) -> [128(k), 2(qc), 128(q)] AP."""
                pTd = sm.tile([128, 16, 128], bf16, name=f"pT_{mt}_{m}", tag="pTs")
                nc.sync.dma_start_transpose(
                    out=pTd, in_=pexp[:, :, :, :, :].rearrange("p a b c d -> p (a b c d)")
                )
                v6 = pTd[:, :, :].rearrange(
                    "p (sl hh qc kc) f -> p sl hh kc qc f", hh=2, qc=2, kc=2
                )
                return lambda sl, hh, kc: v6[:, sl, hh, kc, :, :]

            def emit_av(mt, v_sb, oT, oT_ps, m, sl, hh, rhs_fn):
                t0 = sl * (C // 128)
                tok_sl = slice(sl * C, (sl + 1) * C)
                h = 2 * m + hh
                prow = hh * 64
                for kc in range(2):
                    nc.tensor.matmul(
                        oT_ps[prow : prow + 64, tok_sl],
                        v_sb[:, t0 + kc, h * 64 : (h + 1) * 64],
                        rhs_fn(sl, hh, kc),
                        start=(kc == 0), stop=(kc == 1),
                    )
                if sl == MT_SLICES - 1 and hh == 1:
                    nc.vector.tensor_copy(out=oT[:, m, :], in_=oT_ps)

            def emit_pt_pe(mt, m, sl, hh, pexp, pT_out):
                """Fallback: PE transpose of normalized pexp via identity.
                pT_out layout: [128(k), sl, hh, kc, 256(q)]."""
                pT_ps = ps_att.tile(
                    [128, 2, C], f32, name=f"ptp_{mt}_{m}_{sl}_{hh}", tag="ps_pt", bufs=2
                )
                for qc in range(2):
                    for kc in range(2):
                        nc.tensor.matmul(
                            pT_ps[:, kc, qc * 128 : (qc + 1) * 128],
                            pexp[:, sl, hh, qc, kc * 128 : (kc + 1) * 128],
                            ident16,
                        )
                if hh == 0:
                    nc.vector.tensor_copy(out=pT_out[:, sl, hh, :, :], in_=pT_ps)
                else:
                    nc.scalar.copy(out=pT_out[:, sl, hh, :, :], in_=pT_ps)

            def emit_ffn1_mm(mt, h2T, py, fc, kc):
                nc.tensor.matmul(
                    py, w1_s[:, kc, fc * 128 : (fc + 1) * 128], h2T[:, kc, :, :],
                    start=(kc == 0), stop=(kc == LC - 1),
                )

            def emit_ffn1_relu(mt, py, yT_all, fc):
                nc.scalar.activation(
                    out=yT_all[:, fc, :], in_=py, func=AF.Relu,
                    bias=b1_s[:, fc : fc + 1], scale=1.0,
                )

            def emit_wo_unit(mt, x_sb, oT, xa, t):
                pxa = ps_cyc.tile([128, L], f32, name=f"pxa_{mt}_{t}", tag="ps_cyc")
                for kc in range(LC):
                    nc.tensor.matmul(
                        pxa, oT[:, kc, t * 128 : (t + 1) * 128], wo_s[:, kc, :],
                        start=(kc == 0), stop=(kc == LC - 1),
                    )
                nc.vector.tensor_add(out=xa[:, t, :], in0=pxa, in1=x_sb[:, t, :])

            def emit_ffn2_unit(mt, yT_all, xa, o_sb, t):
                pf = ps_ffn.tile([128, L], f32, name=f"pf_{mt}_{t}", tag="ps_ffn")
                for fc in range(FC):
                    nc.tensor.matmul(
                        pf, yT_all[:, fc, t * 128 : (t + 1) * 128], w2_s[:, fc, :],
                        start=(fc == 0), stop=(fc == FC - 1),
                    )
                nc.vector.tensor_add(out=o_sb[:, t, :], in0=pf, in1=xa[:, t, :])

            def emit_out_store(mt, o_sb):
                nc.gpsimd.dma_start(
                    out=out_v[4 * mt : 4 * mt + 4].rearrange("c p l -> p c l"),
                    in_=o_sb,
                )

            class FFNFiller:
                """Granular FFN1 emission: one matmul per step()."""

                def __init__(self, mt, h2T, yT_all):
                    self.mt = mt
                    self.h2T = h2T
                    self.yT = yT_all
                    self.fc = 0
                    self.kc = 0
                    self.py = None

                def step(self, n):
                    for _ in range(n):
                        if self.fc >= FC:
                            return
                        if self.kc == 0:
                            self.py = ps_ffn.tile(
                                [128, TOK], f32,
                                name=f"py_{self.mt}_{self.fc}", tag="ps_ffn",
                            )
                        emit_ffn1_mm(self.mt, self.h2T, self.py, self.fc, self.kc)
                        self.kc += 1
                        if self.kc == LC:
                            emit_ffn1_relu(self.mt, self.py, self.yT, self.fc)
                            self.kc = 0
                            self.fc += 1

                def finish(self):
                    self.step((FC - self.fc) * LC - self.kc)

            prev = None  # (filler, yT_all, xa) of mt-1
            ln1_state = {}
            qkv_state = {}

            def ensure_ln1(mt):
                if mt not in ln1_state and mt < N_MT:
                    ln1_state[mt] = emit_ln1(mt)

            def ensure_qkv(mt):
                if mt not in qkv_state and mt < N_MT:
                    _, hT_ = ln1_state[mt]
                    qT_, kT_, v_, units_ = emit_qkv_units(mt, hT_)
                    qkv_state[mt] = [qT_, kT_, v_, units_, 0]

            def run_qkv(mt, n, cap=99):
                st = qkv_state.get(mt)
                if st is None:
                    return
                while n > 0 and st[4] < min(len(st[3]), cap):
                    st[3][st[4]]()
                    st[4] += 1
                    n -= 1

            warm_pe()
            load_weights_early()
            load_weights_late()
            ensure_ln1(0)
            for mt in range(N_MT):
                ensure_ln1(mt)
                prefetch_x(mt + 1)
                ensure_qkv(mt)
                if mt == 0:
                    run_qkv(mt, 4)
                else:
                    # weave FFN1-of-(mt-1) into the qkv block so its relus
                    # clear the scalar queue before FFN2 needs yT
                    stq = qkv_state[mt]
                    while stq[4] < len(stq[3]):
                        run_qkv(mt, 1)
                        if prev is not None:
                            prev[0].step(2)
                x_sb, hT = ln1_state.pop(mt)
                qT, kT, v_sb = qkv_state[mt][:3]
                if mt - 1 in qkv_state:
                    del qkv_state[mt - 1]
                oT = act.tile([128, LC, TOK], bf16, name=f"oTs_{mt}", tag="oTs", bufs=1)
                o_prev = None
                if prev is not None:
                    o_prev = outp.tile([128, TC, L], f32, name=f"o_{mt-1}", tag="o")
                pending = None  # delayed AV: (oT_ps, rhs_fn)
                for m in range(LC):
                    oT_ps = ps_cyc.tile([128, TOK], f32, name=f"oT_{mt}_{m}", tag="ps_cyc")
                    pexp = sm.tile(
                        [128, MT_SLICES, 2, 2, C], bf16, name=f"pexp_{mt}_{m}", tag="pexp"
                    )
                    z = stat.tile([128, MT_SLICES, 2, 2], bf16, name=f"z_{mt}_{m}", tag="z")
                    if not ptt_dma:
                        pT_pe = sm.tile(
                            [128, MT_SLICES, 2, 2, C], bf16, name=f"pTf_{mt}_{m}", tag="pTs"
                        )
                    for sl in range(MT_SLICES):
                        sps = {}
                        emit_attn_S(mt, qT, kT, m, sl, sps)
                        emit_soft(mt, m, sl, sps, pexp, z)
                        if prev is not None and m >= (2 if mt == 1 else 1):
                            prev[0].step(3)
                        if mt == 0:
                            run_qkv(0, 2)
                            run_qkv(1, 2, cap=8)
                        if not ptt_dma:
                            emit_pt_pe(mt, m, sl, 0, pexp, pT_pe)
                            emit_pt_pe(mt, m, sl, 1, pexp, pT_pe)
                    if ptt_dma:
                        rhs_fn = emit_pt_dma(mt, m, pexp)
                    else:
                        def rhs_fn(sl, hh, kc, pT_pe=pT_pe):
                            return pT_pe[:, sl, hh, kc, :]
                    if pending is not None:
                        pps, prf = pending
                        for sl in range(MT_SLICES):
                            for hh in range(2):
                                emit_av(mt, v_sb, oT, pps, m - 1, sl, hh, prf)
                            if prev is not None:
                                prev[0].step(3)
                            if mt == 0:
                                run_qkv(1, 2, cap=8)
                    pending = (oT_ps, rhs_fn)
                    if m == 1 and mt == 0:
                        ensure_ln1(mt + 1)
                        ensure_qkv(1)
                pps, prf = pending
                for sl in range(MT_SLICES):
                    for hh in range(2):
                        emit_av(mt, v_sb, oT, pps, LC - 1, sl, hh, prf)
                    if prev is not None:
                        prev[0].step(3)
                if mt == 0:
                    run_qkv(1, 99)  # deferred v units: safe after last AV
                nc.gpsimd.tensor_add(out=x_sb[:, :, :], in0=x_sb[:, :, :], in1=_rep(bo_b, TC))
                if mt >= 1:
                    # ln1(mt+1) here: its hT XBAR transpose runs while the
                    # sync queue is free (no pT transposes pending) and its
                    # DVE work overlaps the wo/ffn2 phase.
                    ensure_ln1(mt + 1)
                xa = act.tile([128, TC, L], f32, name=f"xa_{mt}", tag="xa")
                if prev is not None:
                    prev[0].finish()
                mv2 = stat.tile([128, TC, 2], f32, name=f"mv_ln2_{mt}", tag="mv")
                bn2 = stat.tile([128, 6], f32, name=f"bn_ln2_{mt}", tag="bn")
                for t in range(TC):
                    emit_wo_unit(mt, x_sb, oT, xa, t)
                    emit_stats(xa, mv2, bn2, t)
                    if prev is not None and t < 2:
                        emit_ffn2_unit(mt - 1, prev[1], prev[2], o_prev, t)
                h2T = layernorm_T(xa, "ln2", mt, ln1=False, mv_pre=mv2)
                if prev is not None:
                    for t in (2, 3):
                        emit_ffn2_unit(mt - 1, prev[1], prev[2], o_prev, t)
                    emit_out_store(mt - 1, o_prev)
                nc.gpsimd.tensor_add(out=xa[:, :, :], in0=xa[:, :, :], in1=_rep(b2_b, TC))
                yT_all = yp.tile([128, FC, TOK], bf16, name=f"yT_{mt}", tag="yT")
                prev = (FFNFiller(mt, h2T, yT_all), yT_all, xa)

            prev[0].finish()
            o_last = outp.tile([128, TC, L], f32, name=f"o_{N_MT-1}", tag="o")
            for t in range(TC):
                emit_ffn2_unit(N_MT - 1, prev[1], prev[2], o_last, t)
                nc.gpsimd.dma_start(
                    out=out_v[4 * (N_MT - 1) + t], in_=o_last[:, t, :]
                )

    nc.finalize()
    return nc


def _rep(ap2d, n):
    """[128, L] AP -> [128, n, L] broadcast along a middle dim (step 0)."""
    import concourse.bass as bass

    return bass.AP(
        tensor=ap2d.tensor,
        offset=ap2d.offset,
        ap=[list(ap2d.ap[0]), [0, n]] + [list(d) for d in ap2d.ap[1:]],
    )


def _get_nc():
    lnt = os.environ.get("EEGK_LNT", "dma") == "dma"
    ptt = os.environ.get("EEGK_PTT", "dma") == "dma"
    key = (lnt, ptt)
    if key not in _cache:
        _cache[key] = _build(lnt_dma=lnt, ptt_dma=ptt)
    return _cache[key]


def _install_ntff_shim():
    """Provide antenv.axon_hooks so trace=True works under axon."""
    import types

    if "antenv.axon_hooks" in sys.modules:
        return
    mod = types.ModuleType("antenv.axon_hooks")
    mod._hook = None
    mod.set_axon_ntff_profile_hook = lambda h: setattr(mod, "_hook", h)
    mod.get_axon_ntff_profile_hook = lambda: mod._hook
    sys.modules["antenv.axon_hooks"] = mod
    try:
        import antenv

        antenv.axon_hooks = mod
        from trn_agent_boot import trn_boot

        hook = trn_boot._ntff_profile_via_ctypes("/opt/axon/libaxon_pjrt.so")
        mod.set_axon_ntff_profile_hook(hook)
    except Exception:
        pass


last_exec_ns = None
last_results = None


def kernel(**inputs):
    global last_exec_ns, last_results
    from concourse.bass_utils import run_bass_kernel_spmd
    import ml_dtypes

    bf = ml_dtypes.bfloat16
    nc = _get_nc()

    x = np.asarray(inputs["x"], dtype=np.float32)
    Wq = np.asarray(inputs["Wq"], dtype=np.float32)
    Wk = np.asarray(inputs["Wk"], dtype=np.float32)
    Wv = np.asarray(inputs["Wv"], dtype=np.float32)
    Wo = np.asarray(inputs["Wo"], dtype=np.float32)
    W1 = np.asarray(inputs["W1"], dtype=np.float32)
    W2 = np.asarray(inputs["W2"], dtype=np.float32)
    bo = np.asarray(inputs["bo"], dtype=np.float32)
    b1 = np.asarray(inputs["b1"], dtype=np.float32)
    b2 = np.asarray(inputs["b2"], dtype=np.float32)
    g1 = np.asarray(inputs["g1"], dtype=np.float32)
    be1 = np.asarray(inputs["be1"], dtype=np.float32)
    g2 = np.asarray(inputs["g2"], dtype=np.float32)
    be2 = np.asarray(inputs["be2"], dtype=np.float32)

    def headT(w):  # [H, D, L] -> [L, H*D]
        return np.ascontiguousarray(w.transpose(2, 0, 1).reshape(L, L))

    cq = (Wq @ be1).reshape(L)
    ck = (Wk @ be1).reshape(L)
    cv = (Wv @ be1).reshape(L)
    bop = bo + Wo @ cv
    b1p = b1 + W1 @ be2

    wqkvT = np.concatenate(
        [
            headT(Wq * g1[None, None, :]),
            headT(Wk * g1[None, None, :]),
            headT(Wv * g1[None, None, :]),
        ],
        axis=0,
    )
    wow2T = np.concatenate(
        [np.ascontiguousarray(Wo.T), np.ascontiguousarray(W2.T)], axis=0
    )
    shared = {
        "wqkvT": wqkvT.astype(bf),
        "wow2T": wow2T.astype(bf),
        "w1T": np.ascontiguousarray((W1 * g2[None, :]).T).astype(bf),
        "cqk": np.stack([cq, ck]).astype(np.float32),
        "b1p": b1p.astype(np.float32),
        "bop": bop.astype(np.float32),
        "b2": b2.astype(np.float32),
    }
    x_sl = np.ascontiguousarray(x.reshape(B * S, C, L))
    in_maps = [
        {"x": x_sl[i * SLICES : (i + 1) * SLICES], **shared} for i in range(N_CORES)
    ]

    trace = os.environ.get("EEGK_TRACE", "0") == "1"
    if trace:
        _install_ntff_shim()
    res = run_bass_kernel_spmd(nc, in_maps, core_ids=list(range(N_CORES)), trace=trace)
    last_exec_ns = res.exec_time_ns
    last_results = res
    out = np.concatenate([res.results[i]["out"] for i in range(N_CORES)], axis=0)
    return out.reshape(B, S, C, L).astype(np.float32)



# revision 20
# speedup vs baseline: 1.0505x; 1.0505x over previous
"""EEGFormer transformer-block kernel for 8 Trainium2 NeuronCores.

Strategy: pure data parallelism over the B*S = 128 independent attention
slices; each core processes 16 slices as 8 megatiles of 512 tokens.

v3 (all-bf16 matmuls, fp32 statistics/residuals):
- All transposes (LayerNorm hT/h2T and softmax pT) run on the DMA XBAR
  (dma_start_transpose), freeing ~12k PE cycles per megatile and
  removing their psum->sbuf copies entirely.
- LayerNorm gamma/beta folded into weights/biases host-side.
- rsqrt computed as exp(-0.5*ln(var+eps)): scalar engine never swaps
  activation tables (exp/ln/copy/identity/relu share one table set).
- Weight DMAs ordered behind the first x tile; FFN1 of megatile mt-1 is
  emitted in single-matmul grains to fill every attention-phase stall.
"""

import os
import sys

import numpy as np

if "/opt/trn_rl_repo" not in sys.path and os.path.isdir("/opt/trn_rl_repo"):
    sys.path.insert(0, "/opt/trn_rl_repo")

B, S, C, L = 4, 32, 256, 512
H = 8
D = L // H
FL = 4 * L
EPS = 1e-5
N_CORES = 8
SLICES = (B * S) // N_CORES       # 16 slices per core
MT_SLICES = 2                      # slices per megatile
N_MT = SLICES // MT_SLICES         # 8 megatiles
TOK = C * MT_SLICES                # 512 tokens per megatile
TC = TOK // 128                    # 4 token chunks
LC = L // 128                      # 4 feature chunks
FC = FL // 128                     # 16 ffn-hidden chunks

_cache = {}


def _build(lnt_dma=True, ptt_dma=True):
    import concourse.bacc as bacc
    import concourse.mybir as mybir
    import concourse.tile as tile
    from concourse.masks import make_identity

    f32 = mybir.dt.float32
    bf16 = mybir.dt.bfloat16
    AF = mybir.ActivationFunctionType
    OP = mybir.AluOpType

    nc = bacc.Bacc("TRN2", target_bir_lowering=False)

    x_d = nc.dram_tensor("x", [SLICES, C, L], f32, kind="ExternalInput")
    wqkv_d = nc.dram_tensor("wqkvT", [3 * L, L], bf16, kind="ExternalInput")
    wow2_d = nc.dram_tensor("wow2T", [L + FL, L], bf16, kind="ExternalInput")
    w1_d = nc.dram_tensor("w1T", [L, FL], bf16, kind="ExternalInput")
    cqk_d = nc.dram_tensor("cqk", [2, L], f32, kind="ExternalInput")
    b1_d = nc.dram_tensor("b1p", [FL], f32, kind="ExternalInput")
    bo_d = nc.dram_tensor("bop", [L], f32, kind="ExternalInput")
    b2_d = nc.dram_tensor("b2", [L], f32, kind="ExternalInput")
    out_d = nc.dram_tensor("out", [SLICES, C, L], f32, kind="ExternalOutput")

    x_v = x_d[:, :, :].rearrange("s (tc p) l -> (s tc) p l", p=128)
    out_v = out_d[:, :, :].rearrange("s (tc p) l -> (s tc) p l", p=128)

    import concourse.bass as bass

    def bcast_row(vec_ap, p=128):
        return bass.AP(
            tensor=vec_ap.tensor,
            offset=vec_ap.offset,
            ap=[[0, p]] + list(vec_ap.ap),
        )

    with tile.TileContext(nc) as tc_ctx:
        tc = tc_ctx
        import contextlib

        ctx = contextlib.ExitStack()
        with ctx:
            wpool = ctx.enter_context(tc.tile_pool(name="weights", bufs=1))
            const = ctx.enter_context(tc.tile_pool(name="const", bufs=1))
            xin = ctx.enter_context(tc.tile_pool(name="xin", bufs=2))
            act = ctx.enter_context(tc.tile_pool(name="act", bufs=2))
            sm = ctx.enter_context(tc.tile_pool(name="sm", bufs=3))
            yp = ctx.enter_context(tc.tile_pool(name="yp", bufs=2))
            outp = ctx.enter_context(tc.tile_pool(name="outp", bufs=2))
            stat = ctx.enter_context(tc.tile_pool(name="stat", bufs=12))
            natt = 3 if ptt_dma else 4
            ps_att = ctx.enter_context(tc.tile_pool(name="ps_att", bufs=natt, space="PSUM"))
            ps_ffn = ctx.enter_context(tc.tile_pool(name="ps_ffn", bufs=2, space="PSUM"))
            ps_cyc = ctx.enter_context(tc.tile_pool(name="ps_cyc", bufs=3 if ptt_dma else 2, space="PSUM"))

            ident16 = const.tile([128, 128], bf16)
            make_identity(nc, ident16)

            ones_w = const.tile([128, 128], bf16)
            nc.vector.memset(ones_w, 1.0)

            def warm_pe(n=40):
                # Dummy matmuls at t=0: keep the PE activity window busy
                # while the first DMAs + LN land so HAM un-throttles early
                # and stays warm until the first real matmul (~14us).
                wps = ps_cyc.tile([128, TOK], f32, name="warm", tag="ps_cyc")
                for _ in range(n):
                    nc.tensor.matmul(
                        wps[:, :128], ones_w, ones_w, start=True, stop=True
                    )
                nc.vector.tensor_copy(
                    out=const.tile([128, 128], f32, name="warm_sink"),
                    in_=wps[:, :128],
                )

            wqkv_s = wpool.tile([128, 3, LC, L], bf16)
            wq_s = wqkv_s[:, 0]
            wk_s = wqkv_s[:, 1]
            wv_s = wqkv_s[:, 2]
            wow2_s = wpool.tile([128, LC + FC, L], bf16)
            wo_s = wow2_s[:, :LC]
            w2_s = wow2_s[:, LC:]
            w1_s = wpool.tile([128, LC, FL], bf16)

            cqk_s = const.tile([128, 2, LC], f32)
            cq_s = cqk_s[:, 0]
            ck_s = cqk_s[:, 1]
            b1_s = const.tile([128, FC], f32)
            bo_b = const.tile([128, L], f32)
            b2_b = const.tile([128, L], f32)
            eps_c = const.tile([128, 1], f32)
            nc.vector.memset(eps_c, EPS)
            zero_c = const.tile([128, 1], f32)
            nc.vector.memset(zero_c, 0.0)

            def load_weights_early():
                nc.sync.dma_start(
                    out=wqkv_s,
                    in_=wqkv_d[:, :].rearrange("(w kc p) f -> p w kc f", p=128, w=3),
                )
                nc.sync.dma_start(
                    out=cqk_s,
                    in_=cqk_d[:, :].rearrange("w (c p) -> p w c", p=128),
                )

            def load_weights_late():
                nc.sync.dma_start(
                    out=w1_s, in_=w1_d[:, :].rearrange("(kc p) f -> p kc f", p=128)
                )
                nc.sync.dma_start(
                    out=wow2_s,
                    in_=wow2_d[:, :].rearrange("(kc p) f -> p kc f", p=128),
                )
                nc.sync.dma_start(
                    out=b1_s, in_=b1_d[:].rearrange("(c p) -> p c", p=128)
                )
                nc.gpsimd.dma_start(out=bo_b, in_=bcast_row(bo_d[:]))
                nc.gpsimd.dma_start(out=b2_b, in_=bcast_row(b2_d[:]))

            def emit_stats(x_sb, mv, bn, t):
                nc.vector.bn_stats(out=bn, in_=x_sb[:, t, :])
                nc.vector.bn_aggr(out=mv[:, t, :], in_=bn)

            def layernorm_T(x_sb, tagb, mt, ln1, mv_pre=None):
                """LN over features of x_sb [128, TC, L] fp32 (tokens on
                partitions); returns hT view [128, LC, TC, 128] (features on
                partitions, bf16, no gamma/beta: folded into weights)."""
                xcn = act.tile([128, TC, L], bf16, name=f"xcn_{tagb}_{mt}", tag=f"xcn_{tagb}", bufs=1)
                if mv_pre is not None:
                    mv = mv_pre
                else:
                    mv = stat.tile([128, TC, 2], f32, name=f"mv_{tagb}_{mt}", tag="mv")
                rstd = stat.tile([128, TC], f32, name=f"rstd_{tagb}_{mt}", tag="rstd")
                lnv = stat.tile([128, TC], f32, name=f"lnv_{tagb}_{mt}", tag="lnv")
                bn = stat.tile([128, 6], f32, name=f"bn_{tagb}_{mt}", tag="bn")
                if mv_pre is None:
                    for t in range(TC):
                        emit_stats(x_sb, mv, bn, t)
                # rstd = rsqrt(var+eps) via Newton on DVE (keeps the scalar
                # engine on a single activation table: no ACT_TABLE_LOADs).
                vv = stat.tile([128, TC], f32, name=f"vv_{tagb}_{mt}", tag="vv")
                tt = stat.tile([128, TC], f32, name=f"tt_{tagb}_{mt}", tag="tt")
                nc.vector.tensor_scalar(
                    out=vv, in0=mv[:, :, 1], scalar1=EPS, scalar2=None, op0=OP.add
                )
                # seed y0 = 2/(1+v), then 3 Newton steps y *= 1.5 - 0.5*v*y^2
                nc.vector.tensor_scalar(
                    out=lnv, in0=vv, scalar1=1.0, scalar2=None, op0=OP.add
                )
                nc.vector.reciprocal(out=lnv, in_=lnv)
                nc.vector.tensor_scalar(
                    out=rstd, in0=lnv, scalar1=2.0, scalar2=None, op0=OP.mult
                )
                for _ in range(3):
                    nc.vector.tensor_mul(out=tt, in0=rstd, in1=rstd)
                    nc.vector.tensor_mul(out=tt, in0=tt, in1=vv)
                    nc.vector.tensor_scalar(
                        out=tt, in0=tt, scalar1=-0.5, scalar2=1.5,
                        op0=OP.mult, op1=OP.add,
                    )
                    nc.vector.tensor_mul(out=rstd, in0=rstd, in1=tt)
                for t in range(TC):
                    nc.vector.tensor_scalar(
                        out=xcn[:, t, :], in0=x_sb[:, t, :],
                        scalar1=mv[:, t, 0:1], scalar2=rstd[:, t : t + 1],
                        op0=OP.subtract, op1=OP.mult,
                    )
                if lnt_dma:
                    # hTd[p, t*LC+lc, f] = xcn.T[(t*LC+lc)*128+p, f]
                    hTd = act.tile([128, TC * LC, 128], bf16, name=f"hT_{tagb}_{mt}", tag=f"hT_{tagb}")
                    nc.sync.dma_start_transpose(out=hTd, in_=xcn[:, :, :])
                    return hTd[:, :, :].rearrange("p (t l) f -> p l t f", l=LC)
                hT = act.tile([128, LC, TOK], bf16, name=f"hT_{tagb}_{mt}", tag=f"hT_{tagb}")
                for m in range(LC):
                    hps = ps_cyc.tile([128, TOK], f32, name=f"hps_{tagb}_{mt}_{m}", tag="ps_cyc")
                    for t in range(TC):
                        nc.tensor.matmul(
                            hps[:, t * 128 : (t + 1) * 128],
                            xcn[:, t, m * 128 : (m + 1) * 128],
                            ident16,
                        )
                    if ln1:
                        nc.vector.tensor_copy(out=hT[:, m, :], in_=hps)
                    else:
                        nc.scalar.copy(out=hT[:, m, :], in_=hps)
                return hT[:, :, :].rearrange("p l (t f) -> p l t f", f=128)

            x_pref = {}

            def prefetch_x(mt):
                if mt in x_pref or mt >= N_MT:
                    return
                x_sb = xin.tile([128, TC, L], f32, name=f"x_{mt}", tag="x")
                if mt == 0:
                    for t in range(TC):
                        nc.scalar.dma_start(out=x_sb[:, t, :], in_=x_v[4 * mt + t])
                else:
                    nc.scalar.dma_start(
                        out=x_sb,
                        in_=x_v[4 * mt : 4 * mt + 4].rearrange("c p l -> p c l"),
                    )
                x_pref[mt] = x_sb

            def emit_ln1(mt):
                prefetch_x(mt)
                x_sb = x_pref.pop(mt)
                hT = layernorm_T(x_sb, "ln1", mt, ln1=True)
                return x_sb, hT

            def emit_qkv_units(mt, hT):
                """hT is a [128, LC, TC, 128] view. Returns tiles + closures."""
                qT = act.tile([128, LC, TOK], bf16, name=f"qT_{mt}", tag="qT", bufs=1)
                kT = act.tile([128, LC, TOK], bf16, name=f"kT_{mt}", tag="kT", bufs=1)
                v_sb = act.tile([128, TC, L], bf16, name=f"v_{mt}", tag="v", bufs=1)
                units = []
                for m in range(LC):
                    def mk_q(m=m):
                        pq = ps_cyc.tile([128, TOK], f32, name=f"psq_{mt}_{m}", tag="ps_cyc")
                        for kc in range(LC):
                            nc.tensor.matmul(
                                pq, wq_s[:, kc, m * 128 : (m + 1) * 128],
                                hT[:, kc, :, :],
                                start=(kc == 0), stop=(kc == LC - 1),
                            )
                        nc.vector.tensor_scalar(
                            out=qT[:, m, :], in0=pq,
                            scalar1=cq_s[:, m : m + 1], scalar2=None, op0=OP.add,
                        )
                    def mk_k(m=m):
                        pk = ps_cyc.tile([128, TOK], f32, name=f"psk_{mt}_{m}", tag="ps_cyc")
                        for kc in range(LC):
                            nc.tensor.matmul(
                                pk, wk_s[:, kc, m * 128 : (m + 1) * 128],
                                hT[:, kc, :, :],
                                start=(kc == 0), stop=(kc == LC - 1),
                            )
                        nc.scalar.activation(
                            out=kT[:, m, :], in_=pk, func=AF.Identity,
                            bias=ck_s[:, m : m + 1], scale=1.0,
                        )
                    units.append(mk_q)
                    units.append(mk_k)
                vunits = []
                for t in range(TC):
                    def mk_v(t=t):
                        pv = ps_cyc.tile([128, L], f32, name=f"psv_{mt}_{t}", tag="ps_cyc")
                        for kc in range(LC):
                            nc.tensor.matmul(
                                pv, hT[:, kc, t, :], wv_s[:, kc, :],
                                start=(kc == 0), stop=(kc == LC - 1),
                            )
                        nc.scalar.copy(out=v_sb[:, t, :], in_=pv)
                    vunits.append(mk_v)
                # all q/k first, v last: v-copies of a prefetched megatile
                # must never be emitted before the current megatile's last AV
                # (scalar-queue cycle through the v ring buffer otherwise).
                units = units + vunits
                return qT, kT, v_sb, units

            def emit_attn_S(mt, qT, kT, m, sl, sps):
                tok_sl = slice(sl * C, (sl + 1) * C)
                for hh in range(2):
                    sps[hh] = ps_att.tile(
                        [128, 2, C], f32, name=f"s_{mt}_{m}_{sl}_{hh}", tag="ps_s", bufs=3
                    )
                for qc in range(2):
                    for hh in range(2):
                        prow = hh * 64
                        nc.tensor.matmul(
                            sps[hh][:, qc, :],
                            qT[prow : prow + 64, m, tok_sl][:, qc * 128 : (qc + 1) * 128],
                            kT[prow : prow + 64, m, tok_sl],
                        )

            def emit_soft(mt, m, sl, sps, pexp, z):
                """exp into pexp slice [128, (sl,hh,qc,k)], z-reduce, rz-mul."""
                for hh in range(2):
                    nc.scalar.activation(
                        out=pexp[:, sl, hh, :, :], in_=sps[hh][:, :, :], func=AF.Exp,
                        scale=float(D) ** -0.5, bias=zero_c,
                    )
                with nc.allow_low_precision(reason="softmax z: row-scale only"):
                    for hh in range(2):
                        nc.vector.tensor_reduce(
                            out=z[:, sl, hh, :], in_=pexp[:, sl, hh, :, :],
                            axis=mybir.AxisListType.X, op=OP.add,
                        )
                rz = stat.tile([128, 4], f32, name=f"rz_{mt}_{m}_{sl}", tag="rz")
                nc.vector.reciprocal(out=rz, in_=z[:, sl, :, :].rearrange("p a b -> p (a b)"))
                for hh in range(2):
                    for qc in range(2):
                        nc.vector.tensor_scalar_mul(
                            pexp[:, sl, hh, qc, :], pexp[:, sl, hh, qc, :],
                            rz[:, 2 * hh + qc : 2 * hh + qc + 1],
                        )

            def emit_pt_dma(mt, m, pexp):
                """One XBAR transpose for the whole [128, 2048] pexp of m.
                pTd[p, mm, f] = pexp[q=f, free=mm*128+p]; mm = sl*8+hh*4+qc*2+kc.
                Returns rhs_fn(sl, hh, kc) -> [128(k), 2(qc), 128(q)] AP."""
                pTd = sm.tile([128, 16, 128], bf16, name=f"pT_{mt}_{m}", tag="pTs")
                nc.sync.dma_start_transpose(
                    out=pTd, in_=pexp[:, :, :, :, :].rearrange("p a b c d -> p (a b c d)")
                )
                v6 = pTd[:, :, :].rearrange(
                    "p (sl hh qc kc) f -> p sl hh kc qc f", hh=2, qc=2, kc=2
                )
                return lambda sl, hh, kc: v6[:, sl, hh, kc, :, :]

            def emit_av(mt, v_sb, oT, oT_ps, m, sl, hh, rhs_fn):
                t0 = sl * (C // 128)
                tok_sl = slice(sl * C, (sl + 1) * C)
                h = 2 * m + hh
                prow = hh * 64
                for kc in range(2):
                    nc.tensor.matmul(
                        oT_ps[prow : prow + 64, tok_sl],
                        v_sb[:, t0 + kc, h * 64 : (h + 1) * 64],
                        rhs_fn(sl, hh, kc),
                        start=(kc == 0), stop=(kc == 1),
                    )
                if sl == MT_SLICES - 1 and hh == 1:
                    nc.scalar.copy(out=oT[:, m, :], in_=oT_ps)

            def emit_pt_pe(mt, m, sl, hh, pexp, pT_out):
                """Fallback: PE transpose of normalized pexp via identity.
                pT_out layout: [128(k), sl, hh, kc, 256(q)]."""
                pT_ps = ps_att.tile(
                    [128, 2, C], f32, name=f"ptp_{mt}_{m}_{sl}_{hh}", tag="ps_pt", bufs=2
                )
                for qc in range(2):
                    for kc in range(2):
                        nc.tensor.matmul(
                            pT_ps[:, kc, qc * 128 : (qc + 1) * 128],
                            pexp[:, sl, hh, qc, kc * 128 : (kc + 1) * 128],
                            ident16,
                        )
                if hh == 0:
                    nc.vector.tensor_copy(out=pT_out[:, sl, hh, :, :], in_=pT_ps)
                else:
                    nc.scalar.copy(out=pT_out[:, sl, hh, :, :], in_=pT_ps)

            def emit_ffn1_mm(mt, h2T, py, fc, kc):
                nc.tensor.matmul(
                    py, w1_s[:, kc, fc * 128 : (fc + 1) * 128], h2T[:, kc, :, :],
                    start=(kc == 0), stop=(kc == LC - 1),
                )

            def emit_ffn1_relu(mt, py, yT_all, fc):
                nc.scalar.activation(
                    out=yT_all[:, fc, :], in_=py, func=AF.Relu,
                    bias=b1_s[:, fc : fc + 1], scale=1.0,
                )

            def emit_wo_unit(mt, x_sb, oT, xa, t):
                pxa = ps_cyc.tile([128, L], f32, name=f"pxa_{mt}_{t}", tag="ps_cyc")
                for kc in range(LC):
                    nc.tensor.matmul(
                        pxa, oT[:, kc, t * 128 : (t + 1) * 128], wo_s[:, kc, :],
                        start=(kc == 0), stop=(kc == LC - 1),
                    )
                nc.vector.tensor_add(out=xa[:, t, :], in0=pxa, in1=x_sb[:, t, :])

            def emit_ffn2_unit(mt, yT_all, xa, o_sb, t):
                pf = ps_ffn.tile([128, L], f32, name=f"pf_{mt}_{t}", tag="ps_ffn")
                for fc in range(FC):
                    nc.tensor.matmul(
                        pf, yT_all[:, fc, t * 128 : (t + 1) * 128], w2_s[:, fc, :],
                        start=(fc == 0), stop=(fc == FC - 1),
                    )
                nc.vector.tensor_add(out=o_sb[:, t, :], in0=pf, in1=xa[:, t, :])

            def emit_out_store(mt, o_sb):
                nc.gpsimd.dma_start(
                    out=out_v[4 * mt : 4 * mt + 4].rearrange("c p l -> p c l"),
                    in_=o_sb,
                )

            class FFNFiller:
                """Granular FFN1 emission: one matmul per step()."""

                def __init__(self, mt, h2T, yT_all):
                    self.mt = mt
                    self.h2T = h2T
                    self.yT = yT_all
                    self.fc = 0
                    self.kc = 0
                    self.py = None

                def step(self, n):
                    for _ in range(n):
                        if self.fc >= FC:
                            return
                        if self.kc == 0:
                            self.py = ps_ffn.tile(
                                [128, TOK], f32,
                                name=f"py_{self.mt}_{self.fc}", tag="ps_ffn",
                            )
                        emit_ffn1_mm(self.mt, self.h2T, self.py, self.fc, self.kc)
                        self.kc += 1
                        if self.kc == LC:
                            emit_ffn1_relu(self.mt, self.py, self.yT, self.fc)
                            self.kc = 0
                            self.fc += 1

                def finish(self):
                    self.step((FC - self.fc) * LC - self.kc)

            prev = None  # (filler, yT_all, xa) of mt-1
            ln1_state = {}
            qkv_state = {}

            def ensure_ln1(mt):
                if mt not in ln1_state and mt < N_MT:
                    ln1_state[mt] = emit_ln1(mt)

            def ensure_qkv(mt):
                if mt not in qkv_state and mt < N_MT:
                    _, hT_ = ln1_state[mt]
                    qT_, kT_, v_, units_ = emit_qkv_units(mt, hT_)
                    qkv_state[mt] = [qT_, kT_, v_, units_, 0]

            def run_qkv(mt, n, cap=99):
                st = qkv_state.get(mt)
                if st is None:
                    return
                while n > 0 and st[4] < min(len(st[3]), cap):
                    st[3][st[4]]()
                    st[4] += 1
                    n -= 1

            warm_pe()
            load_weights_early()
            load_weights_late()
            ensure_ln1(0)
            for mt in range(N_MT):
                ensure_ln1(mt)
                prefetch_x(mt + 1)
                ensure_qkv(mt)
                if mt == 0:
                    run_qkv(mt, 4)
                else:
                    # weave FFN1-of-(mt-1) into the qkv block so its relus
                    # clear the scalar queue before FFN2 needs yT
                    stq = qkv_state[mt]
                    while stq[4] < len(stq[3]):
                        run_qkv(mt, 1)
                        if prev is not None:
                            prev[0].step(2)
                x_sb, hT = ln1_state.pop(mt)
                qT, kT, v_sb = qkv_state[mt][:3]
                if mt - 1 in qkv_state:
                    del qkv_state[mt - 1]
                oT = act.tile([128, LC, TOK], bf16, name=f"oTs_{mt}", tag="oTs", bufs=1)
                o_prev = None
                if prev is not None:
                    o_prev = outp.tile([128, TC, L], f32, name=f"o_{mt-1}", tag="o")
                pending = None  # delayed AV: (oT_ps, rhs_fn)
                for m in range(LC):
                    oT_ps = ps_cyc.tile([128, TOK], f32, name=f"oT_{mt}_{m}", tag="ps_cyc")
                    pexp = sm.tile(
                        [128, MT_SLICES, 2, 2, C], bf16, name=f"pexp_{mt}_{m}", tag="pexp"
                    )
                    z = stat.tile([128, MT_SLICES, 2, 2], bf16, name=f"z_{mt}_{m}", tag="z")
                    if not ptt_dma:
                        pT_pe = sm.tile(
                            [128, MT_SLICES, 2, 2, C], bf16, name=f"pTf_{mt}_{m}", tag="pTs"
                        )
                    for sl in range(MT_SLICES):
                        sps = {}
                        emit_attn_S(mt, qT, kT, m, sl, sps)
                        emit_soft(mt, m, sl, sps, pexp, z)
                        if prev is not None and m >= (2 if mt == 1 else 1):
                            prev[0].step(3)
                        if mt == 0:
                            run_qkv(0, 2)
                            run_qkv(1, 2, cap=8)
                        if not ptt_dma:
                            emit_pt_pe(mt, m, sl, 0, pexp, pT_pe)
                            emit_pt_pe(mt, m, sl, 1, pexp, pT_pe)
                    if ptt_dma:
                        rhs_fn = emit_pt_dma(mt, m, pexp)
                    else:
                        def rhs_fn(sl, hh, kc, pT_pe=pT_pe):
                            return pT_pe[:, sl, hh, kc, :]
                    if pending is not None:
                        pps, prf = pending
                        for sl in range(MT_SLICES):
                            for hh in range(2):
                                emit_av(mt, v_sb, oT, pps, m - 1, sl, hh, prf)
                            if prev is not None:
                                prev[0].step(3)
                            if mt == 0:
                                run_qkv(1, 2, cap=8)
                    pending = (oT_ps, rhs_fn)
                    if m == 1 and mt == 0:
                        ensure_ln1(mt + 1)
                        ensure_qkv(1)
                pps, prf = pending
                for sl in range(MT_SLICES):
                    for hh in range(2):
                        emit_av(mt, v_sb, oT, pps, LC - 1, sl, hh, prf)
                    if prev is not None:
                        prev[0].step(3)
                if mt == 0:
                    run_qkv(1, 99)  # deferred v units: safe after last AV
                nc.gpsimd.tensor_add(out=x_sb[:, :, :], in0=x_sb[:, :, :], in1=_rep(bo_b, TC))
                if mt >= 1:
                    # ln1(mt+1) here: its hT XBAR transpose runs while the
                    # sync queue is free (no pT transposes pending) and its
                    # DVE work overlaps the wo/ffn2 phase.
                    ensure_ln1(mt + 1)
                xa = act.tile([128, TC, L], f32, name=f"xa_{mt}", tag="xa")
                if prev is not None:
                    prev[0].finish()
                mv2 = stat.tile([128, TC, 2], f32, name=f"mv_ln2_{mt}", tag="mv")
                bn2 = stat.tile([128, 6], f32, name=f"bn_ln2_{mt}", tag="bn")
                for t in range(TC):
                    emit_wo_unit(mt, x_sb, oT, xa, t)
                    emit_stats(xa, mv2, bn2, t)
                    if prev is not None and t < 2:
                        emit_ffn2_unit(mt - 1, prev[1], prev[2], o_prev, t)
                h2T = layernorm_T(xa, "ln2", mt, ln1=False, mv_pre=mv2)
                if prev is not None:
                    for t in (2, 3):
                        emit_ffn2_unit(mt - 1, prev[1], prev[2], o_prev, t)
                    emit_out_store(mt - 1, o_prev)
                nc.gpsimd.tensor_add(out=xa[:, :, :], in0=xa[:, :, :], in1=_rep(b2_b, TC))
                yT_all = yp.tile([128, FC, TOK], bf16, name=f"yT_{mt}", tag="yT")
                prev = (FFNFiller(mt, h2T, yT_all), yT_all, xa)

            prev[0].finish()
            o_last = outp.tile([128, TC, L], f32, name=f"o_{N_MT-1}", tag="o")
            for t in range(TC):
                emit_ffn2_unit(N_MT - 1, prev[1], prev[2], o_last, t)
                nc.gpsimd.dma_start(
                    out=out_v[4 * (N_MT - 1) + t], in_=o_last[:, t, :]
                )

    nc.finalize()
    return nc


def _rep(ap2d, n):
    """[128, L] AP -> [128, n, L] broadcast along a middle dim (step 0)."""
    import concourse.bass as bass

    return bass.AP(
        tensor=ap2d.tensor,
        offset=ap2d.offset,
        ap=[list(ap2d.ap[0]), [0, n]] + [list(d) for d in ap2d.ap[1:]],
    )


def _get_nc():
    lnt = os.environ.get("EEGK_LNT", "dma") == "dma"
    ptt = os.environ.get("EEGK_PTT", "dma") == "dma"
    key = (lnt, ptt)
    if key not in _cache:
        _cache[key] = _build(lnt_dma=lnt, ptt_dma=ptt)
    return _cache[key]


def _install_ntff_shim():
    """Provide antenv.axon_hooks so trace=True works under axon."""
    import types

    if "antenv.axon_hooks" in sys.modules:
        return
    mod = types.ModuleType("antenv.axon_hooks")
    mod._hook = None
    mod.set_axon_ntff_profile_hook = lambda h: setattr(mod, "_hook", h)
    mod.get_axon_ntff_profile_hook = lambda: mod._hook
    sys.modules["antenv.axon_hooks"] = mod
    try:
        import antenv

        antenv.axon_hooks = mod
        from trn_agent_boot import trn_boot

        hook = trn_boot._ntff_profile_via_ctypes("/opt/axon/libaxon_pjrt.so")
        mod.set_axon_ntff_profile_hook(hook)
    except Exception:
        pass


last_exec_ns = None
last_results = None


def kernel(**inputs):
    global last_exec_ns, last_results
    from concourse.bass_utils import run_bass_kernel_spmd
    import ml_dtypes

    bf = ml_dtypes.bfloat16
    nc = _get_nc()

    x = np.asarray(inputs["x"], dtype=np.float32)
    Wq = np.asarray(inputs["Wq"], dtype=np.float32)
    Wk = np.asarray(inputs["Wk"], dtype=np.float32)
    Wv = np.asarray(inputs["Wv"], dtype=np.float32)
    Wo = np.asarray(inputs["Wo"], dtype=np.float32)
    W1 = np.asarray(inputs["W1"], dtype=np.float32)
    W2 = np.asarray(inputs["W2"], dtype=np.float32)
    bo = np.asarray(inputs["bo"], dtype=np.float32)
    b1 = np.asarray(inputs["b1"], dtype=np.float32)
    b2 = np.asarray(inputs["b2"], dtype=np.float32)
    g1 = np.asarray(inputs["g1"], dtype=np.float32)
    be1 = np.asarray(inputs["be1"], dtype=np.float32)
    g2 = np.asarray(inputs["g2"], dtype=np.float32)
    be2 = np.asarray(inputs["be2"], dtype=np.float32)

    def headT(w):  # [H, D, L] -> [L, H*D]
        return np.ascontiguousarray(w.transpose(2, 0, 1).reshape(L, L))

    cq = (Wq @ be1).reshape(L)
    ck = (Wk @ be1).reshape(L)
    cv = (Wv @ be1).reshape(L)
    bop = bo + Wo @ cv
    b1p = b1 + W1 @ be2

    wqkvT = np.concatenate(
        [
            headT(Wq * g1[None, None, :]),
            headT(Wk * g1[None, None, :]),
            headT(Wv * g1[None, None, :]),
        ],
        axis=0,
    )
    wow2T = np.concatenate(
        [np.ascontiguousarray(Wo.T), np.ascontiguousarray(W2.T)], axis=0
    )
    shared = {
        "wqkvT": wqkvT.astype(bf),
        "wow2T": wow2T.astype(bf),
        "w1T": np.ascontiguousarray((W1 * g2[None, :]).T).astype(bf),
        "cqk": np.stack([cq, ck]).astype(np.float32),
        "b1p": b1p.astype(np.float32),
        "bop": bop.astype(np.float32),
        "b2": b2.astype(np.float32),
    }
    x_sl = np.ascontiguousarray(x.reshape(B * S, C, L))
    in_maps = [
        {"x": x_sl[i * SLICES : (i + 1) * SLICES], **shared} for i in range(N_CORES)
    ]

    trace = os.environ.get("EEGK_TRACE", "0") == "1"
    if trace:
        _install_ntff_shim()
    res = run_bass_kernel_spmd(nc, in_maps, core_ids=list(range(N_CORES)), trace=trace)
    last_exec_ns = res.exec_time_ns
    last_results = res
    out = np.concatenate([res.results[i]["out"] for i in range(N_CORES)], axis=0)
    return out.reshape(B, S, C, L).astype(np.float32)

